# revision 15
# baseline (speedup 1.0000x reference)
"""Sigmoid-attention block on 8 TRN2 NeuronCores, v2.

Sharding: core c = (batch b=c//2, head-half hh=c%2).  Each core computes
Q^T/K^T directly in transposed layout (W^T @ x^T, no PE transposes), ropes
them with partition-shifted DVE ops, runs causal sigmoid attention for its
6 heads with query-window-restricted diagonal chunks (no work on fully
masked regions), and computes HALF of the epilogue: per-qb partial LN
stats are exchanged in a tiny AllGather, each core LayerNorms + gates only
its own 384-column half, the gated halves are AllGathered, and the output
projection produces the core's own 384 output columns.

Emission interleaves projection seq-blocks with attention query-blocks so
the ScalarE sigmoid stream (the co-bottleneck) starts early.
"""

import numpy as np
import ml_dtypes

import concourse.bass as bass
import concourse.bacc as bacc
import concourse.mybir as mybir
import concourse.tile as tile
from concourse import bass_utils

BF16 = mybir.dt.bfloat16
F32 = mybir.dt.float32
AF = mybir.ActivationFunctionType

S = 2048          # sequence length
HID = 768         # hidden
NH = 6            # heads per core
NPAIR = 3         # head pairs per core
D = 64            # head dim
RB = 512          # row block (query block size)
NQB = 4           # query blocks
LN_EPS = 1e-8
N_CORES = 8


def _rope_tables():
    inv_freq = 1.0 / (10000.0 ** (np.arange(0, D, 2, dtype=np.float64) / D))
    t = np.arange(S, dtype=np.float64)
    freqs = np.outer(t, inv_freq)                      # [S, 32]
    emb = np.concatenate([freqs, freqs], axis=-1)      # [S, 64]
    return np.cos(emb).astype(np.float32), np.sin(emb).astype(np.float32)


_DEBUG = False     # adds slab-dump outputs (dev only)


def build_nc(ndev, pairs):
    nc = bacc.Bacc("TRN2", target_bir_lowering=False, debug=False,
                   num_devices=ndev)

    def din(name, shape, dt):
        return nc.dram_tensor(name, shape, dt, kind="ExternalInput").ap()

    xT = din("xT", [HID, S], BF16)
    w_qku = din("w_qku", [HID, 1152], BF16)        # Q(384) | K(384) | U(384)
    wv = din("wv", [HID, 384], BF16)
    w_out = din("w_out", [HID, 384], BF16)         # gamma-folded, own cols
    cosT2 = din("cosT2", [128, S], BF16)           # cos^T stacked 2x
    sinfT2 = din("sinfT2", [128, S], BF16)         # sign-folded sin^T 2x
    maskf = din("maskf", [128, RB], BF16)          # (col >= row)
    maskd2 = din("maskd2", [128, 384], BF16)       # [mf[:,256:512] | mf[:,384:512]]
    ones_k = din("ones_k", [128, 1], BF16)
    residT = din("residT", [384, S], F32)          # x^T half + b_out
    out = nc.dram_tensor("out", [384, S], F32, kind="ExternalOutput").ap()

    xT_r = xT.rearrange("(k p) s -> p k s", p=128)          # [128, 6, S]
    wqku_r = w_qku.rearrange("(k p) c -> p k c", p=128)     # [128, 6, 1152]
    wv_r = wv.rearrange("(k p) c -> p k c", p=128)          # [128, 6, 384]
    wout_r = w_out.rearrange("(k p) c -> p k c", p=128)     # [128, 6, 384]
    residT_r = residT.rearrange("(c p) s -> p c s", p=128)  # [128, 3, S]
    out_r = out.rearrange("(c p) s -> p c s", p=128)

    dbg = None
    if _DEBUG:
        dbg = {n: nc.dram_tensor(n, sh, BF16, kind="ExternalOutput").ap()
               for n, sh in [("qt_d", [128, 3, S]), ("kt_d", [128, 3, S]),
                             ("v_d", [128, 16, 384]), ("ut_d", [128, 3, S]),
                             ("ao_d", [128, 3, S])]}
    with tile.TileContext(nc) as tc:
        _emit(nc, tc, pairs, xT_r, wqku_r, wv_r, wout_r, cosT2, sinfT2,
              maskf, maskd2, ones_k, residT_r, out_r, dbg)
    nc.compile()
    return nc


def _emit(nc, tc, pairs, xT_r, wqku_r, wv_r, wout_r, cosT2, sinfT2,
          maskf, maskd2, ones_k, residT_r, out_r, dbg=None):
    from contextlib import ExitStack
    es = ExitStack()
    with es:
        # ---- resident SBUF tensors -----------------------------------
        res = es.enter_context(tc.tile_pool(name="resident", bufs=1))
        xT_sb = res.tile([128, 6, S], BF16, tag="xT")
        wqku_sb = res.tile([128, 6, 1152], BF16, tag="wqku")
        wv_sb = res.tile([128, 6, 384], BF16, tag="wv")
        wout_sb = res.tile([128, 6, 384], BF16, tag="wout")
        cos_sb = res.tile([128, S], BF16, tag="cos")
        sinf_sb = res.tile([128, S], BF16, tag="sinf")
        maskf_sb = res.tile([128, RB], BF16, tag="maskf")
        maskd2_sb = res.tile([128, 384], BF16, tag="maskd2")
        ones_k_sb = res.tile([128, 1], BF16, tag="onesk")
        qt_sb = res.tile([128, NPAIR, S], BF16, tag="qt")   # roped Q^T
        kt_sb = res.tile([128, NPAIR, S], BF16, tag="kt")   # roped K^T
        v_sb = res.tile([128, 16, 384], BF16, tag="v")      # V row layout
        ut_sb = res.tile([128, 3, S], BF16, tag="ut")       # silu(U)^T half
        ao_sb = res.tile([128, 3, S], BF16, tag="ao")       # attn out^T half

        # load order: first seq-block + K weights first so the first
        # matmuls start early
        for k in range(6):
            nc.sync.dma_start(out=xT_sb[:, k, 0:RB], in_=xT_r[:, k, 0:RB])
        for k in range(6):
            nc.sync.dma_start(out=wqku_sb[:, k, :], in_=wqku_r[:, k, :])
        nc.sync.dma_start(out=cos_sb[:, 0:RB], in_=cosT2[:, 0:RB])
        nc.sync.dma_start(out=sinf_sb[:, 0:RB], in_=sinfT2[:, 0:RB])
        for k in range(6):
            nc.sync.dma_start(out=wv_sb[:, k, :], in_=wv_r[:, k, :])
        nc.sync.dma_start(out=maskf_sb[:], in_=maskf[:])
        nc.sync.dma_start(out=maskd2_sb[:], in_=maskd2[:])
        nc.sync.dma_start(out=ones_k_sb[:], in_=ones_k[:])
        for nb in range(1, 4):
            sl = slice(nb * RB, (nb + 1) * RB)
            for k in range(6):
                nc.sync.dma_start(out=xT_sb[:, k, sl], in_=xT_r[:, k, sl])
            nc.sync.dma_start(out=cos_sb[:, sl], in_=cosT2[:, sl])
            nc.sync.dma_start(out=sinf_sb[:, sl], in_=sinfT2[:, sl])
        for k in range(6):
            nc.sync.dma_start(out=wout_sb[:, k, :], in_=wout_r[:, k, :])

        # ---- pools ---------------------------------------------------
        scp = es.enter_context(tc.tile_pool(name="scp", bufs=2,
                                            space="PSUM"))   # 4 banks
        avp = es.enter_context(tc.tile_pool(name="avp", bufs=1,
                                            space="PSUM"))   # 1 bank
        atp = es.enter_context(tc.tile_pool(name="atp", bufs=16))
        sb1 = es.enter_context(tc.tile_pool(name="p1sb", bufs=2))
        dram = es.enter_context(tc.tile_pool(name="agdram", bufs=2,
                                             space="DRAM"))

        # ------------- phase 1 helpers --------------------------------
        def proj_rope(pp, role, p, nb):
            """role 0=Q, 1=K: project pair p's 128 ^T-rows for seq block
            nb and rope into qt/kt."""
            sl = slice(nb * RB, (nb + 1) * RB)
            pq = pp.tile([128, RB], F32, tag="pp")
            c0 = role * 384 + p * 128
            for k in range(6):
                nc.tensor.matmul(pq[:], wqku_sb[:, k, c0:c0 + 128],
                                 xT_sb[:, k, sl], start=(k == 0),
                                 stop=(k == 5))
            # rope: w = pq*g (aligned), shift w across 32-blocks (single-
            # input copies -- the only partition-base-mismatch the HW
            # verifier allows), out = pq*cos + shifted(w)
            w = sb1.tile([128, RB], BF16, tag="w")
            ws = sb1.tile([128, RB], BF16, tag="ws")
            t2 = sb1.tile([128, RB], BF16, tag="t2")
            nc.vector.tensor_mul(w[:], pq[:], sinf_sb[:, sl])
            nc.vector.tensor_copy(ws[0:32, :], w[32:64, :])
            nc.vector.tensor_copy(ws[32:64, :], w[0:32, :])
            nc.vector.tensor_copy(ws[64:96, :], w[96:128, :])
            nc.vector.tensor_copy(ws[96:128, :], w[64:96, :])
            nc.vector.tensor_mul(t2[:], pq[:], cos_sb[:, sl])
            dst = kt_sb if role else qt_sb
            nc.vector.tensor_add(dst[:, p, sl], t2[:], ws[:])

        def proj_u(pp, ct, nb):
            sl = slice(nb * RB, (nb + 1) * RB)
            pu = pp.tile([128, RB], F32, tag="pp", name="pu")
            c0 = 768 + ct * 128
            for k in range(6):
                nc.tensor.matmul(pu[:], wqku_sb[:, k, c0:c0 + 128],
                                 xT_sb[:, k, sl], start=(k == 0),
                                 stop=(k == 5))
            usig = sb1.tile([128, RB], BF16, tag="usig")
            nc.scalar.activation(usig[:], pu[:], AF.Sigmoid)
            nc.vector.tensor_mul(ut_sb[:, ct, sl], usig[:], pu[:])

        def proj_v(pp, rt):
            pv = pp.tile([128, RB], F32, tag="pp", name="pv")
            for k in range(6):
                nc.tensor.matmul(pv[:, 0:384],
                                 xT_sb[:, k, rt * 128:(rt + 1) * 128],
                                 wv_sb[:, k, :], start=(k == 0), stop=(k == 5))
            nc.vector.tensor_copy(v_sb[:, rt, :], pv[:, 0:384])

        def proj_block(pp, nb):
            for p in range(NPAIR):
                proj_rope(pp, 1, p, nb)      # K first
            for p in range(NPAIR):
                proj_rope(pp, 0, p, nb)
            for rt in range(4 * nb, 4 * nb + 4):
                proj_v(pp, rt)
            for ct in range(3):
                proj_u(pp, ct, nb)

        # ------------- attention --------------------------------------
        def attn_pair(qb, p):
            q0 = qb * RB
            ats = []                          # (at, h0col, avcol, n)
            for kc in range(4 * qb):          # fully unmasked chunks
                sc = scp.tile([128, 1024], F32, tag="sc")
                at = atp.tile([128, 1024], BF16, tag="at")
                for h in range(2):
                    b0 = 64 * h
                    nc.tensor.matmul(
                        sc[:, h * RB:(h + 1) * RB],
                        kt_sb[b0:b0 + 64, p, kc * 128:(kc + 1) * 128],
                        qt_sb[b0:b0 + 64, p, q0:q0 + RB],
                        start=True, stop=True)
                nc.scalar.activation(at[:], sc[:], AF.Sigmoid, scale=0.125)
                ats.append((at, RB, 0, RB))
            # diagonal chunks t=0..3: query windows 512/384/256/128
            kcd = 4 * qb
            # D0: t=0, full window, one [128,1024] tile like nondiag
            sc = scp.tile([128, 1024], F32, tag="sc", name="scd0")
            at0 = atp.tile([128, 1024], BF16, tag="at", name="atd0")
            for h in range(2):
                b0 = 64 * h
                nc.tensor.matmul(
                    sc[:, h * RB:(h + 1) * RB],
                    kt_sb[b0:b0 + 64, p, kcd * 128:(kcd + 1) * 128],
                    qt_sb[b0:b0 + 64, p, q0:q0 + RB],
                    start=True, stop=True)
            nc.scalar.activation(at0[:], sc[:], AF.Sigmoid, scale=0.125)
            for h in range(2):
                nc.vector.tensor_mul(at0[:, h * RB:(h + 1) * RB],
                                     at0[:, h * RB:(h + 1) * RB], maskf_sb[:])
            # D1: t=1, window [128,512): per-head 384 cols at h*512
            sc1 = scp.tile([128, 1024], F32, tag="sc", name="scd1")
            at1 = atp.tile([128, 1024], BF16, tag="at", name="atd1")
            for h in range(2):
                b0 = 64 * h
                nc.tensor.matmul(
                    sc1[:, h * RB:h * RB + 384],
                    kt_sb[b0:b0 + 64, p, (kcd + 1) * 128:(kcd + 2) * 128],
                    qt_sb[b0:b0 + 64, p, q0 + 128:q0 + RB],
                    start=True, stop=True)
                nc.scalar.activation(at1[:, h * RB:h * RB + 384],
                                     sc1[:, h * RB:h * RB + 384],
                                     AF.Sigmoid, scale=0.125)
                nc.vector.tensor_mul(at1[:, h * RB:h * RB + 384],
                                     at1[:, h * RB:h * RB + 384],
                                     maskf_sb[:, 0:384])
            # D2: t=2 (N=256) + t=3 (N=128): per-head 384 cols at h*512
            sc2 = scp.tile([128, 1024], F32, tag="sc", name="scd2")
            at2 = atp.tile([128, 1024], BF16, tag="at", name="atd2")
            for h in range(2):
                b0 = 64 * h
                nc.tensor.matmul(
                    sc2[:, h * RB:h * RB + 256],
                    kt_sb[b0:b0 + 64, p, (kcd + 2) * 128:(kcd + 3) * 128],
                    qt_sb[b0:b0 + 64, p, q0 + 256:q0 + RB],
                    start=True, stop=True)
                nc.tensor.matmul(
                    sc2[:, h * RB + 256:h * RB + 384],
                    kt_sb[b0:b0 + 64, p, (kcd + 3) * 128:(kcd + 4) * 128],
                    qt_sb[b0:b0 + 64, p, q0 + 384:q0 + RB],
                    start=True, stop=True)
                nc.scalar.activation(at2[:, h * RB:h * RB + 384],
                                     sc2[:, h * RB:h * RB + 384],
                                     AF.Sigmoid, scale=0.125)
                nc.vector.tensor_mul(at2[:, h * RB:h * RB + 384],
                                     at2[:, h * RB:h * RB + 384],
                                     maskd2_sb[:])
            # ---- A @ V ------------------------------------------------
            av = avp.tile([128, RB], F32, tag="av")
            for kc in range(4 * qb):
                at = ats[kc][0]
                for h in range(2):
                    b0 = 64 * h
                    nc.tensor.matmul(
                        av[b0:b0 + 64, :],
                        v_sb[:, kc, (2 * p + h) * 64:(2 * p + h + 1) * 64],
                        at[:, h * RB:(h + 1) * RB],
                        start=(kc == 0), stop=False, skip_group_check=True)
            for h in range(2):
                b0 = 64 * h
                vh = lambda kc: v_sb[:, kc, (2 * p + h) * 64:(2 * p + h + 1) * 64]
                nc.tensor.matmul(av[b0:b0 + 64, :], vh(kcd),
                                 at0[:, h * RB:(h + 1) * RB],
                                 start=(qb == 0), stop=False,
                                 skip_group_check=True)
                nc.tensor.matmul(av[b0:b0 + 64, 128:RB], vh(kcd + 1),
                                 at1[:, h * RB:h * RB + 384],
                                 start=False, stop=False, skip_group_check=True)
                nc.tensor.matmul(av[b0:b0 + 64, 256:RB], vh(kcd + 2),
                                 at2[:, h * RB:h * RB + 256],
                                 start=False, stop=False, skip_group_check=True)
                nc.tensor.matmul(av[b0:b0 + 64, 384:RB], vh(kcd + 3),
                                 at2[:, h * RB + 256:h * RB + 384],
                                 start=False, stop=True, skip_group_check=True)
            nc.vector.tensor_copy(ao_sb[:, p, q0:q0 + RB], av[:])

        def attn_block(qb):
            for p in range(NPAIR):
                attn_pair(qb, p)

        # ------------- epilogue ---------------------------------------
        def epilogue(qb, stp, opo, sb3, ssb):
            q0 = qb * RB
            sl = slice(q0, q0 + RB)
            st = stp.tile([128, RB], F32, tag="st")     # row0 ssum, row32 qsum
            for ct in range(3):
                sq = sb3.tile([128, RB], BF16, tag="sq")
                nc.vector.tensor_mul(sq[:], ao_sb[:, ct, sl], ao_sb[:, ct, sl])
                nc.tensor.matmul(st[0:1, :], ones_k_sb[:], ao_sb[:, ct, sl],
                                 start=(ct == 0), stop=(ct == 2),
                                 skip_group_check=True)
                nc.tensor.matmul(st[32:33, :], ones_k_sb[:], sq[:],
                                 start=(ct == 0), stop=(ct == 2),
                                 skip_group_check=True)
            stsb = sb3.tile([1, 2 * RB], F32, tag="stsb")
            nc.vector.tensor_copy(stsb[:, 0:RB], st[0:1, :])
            nc.vector.tensor_copy(stsb[:, RB:2 * RB], st[32:33, :])
            stin = dram.tile([1, 2 * RB], F32, tag="stin")
            stout = dram.tile([1, 2, 2 * RB], F32, tag="stout")
            nc.gpsimd.dma_start(out=stin[:], in_=stsb[:])
            nc.gpsimd.collective_compute(
                "AllGather", mybir.AluOpType.bypass, replica_groups=pairs,
                ins=[stin.opt()], outs=[stout.opt()])
            stf = sb3.tile([1, 2, 2 * RB], F32, tag="stf")
            nc.sync.dma_start(out=stf[:], in_=stout[:])
            mv = ssb.tile([1, 2 * RB], F32, tag="mv")
            nc.vector.tensor_add(mv[:], stf[:, 0, :], stf[:, 1, :])
            nc.vector.tensor_scalar_mul(mv[:], mv[:], 1.0 / HID)
            musq = ssb.tile([1, RB], F32, tag="musq")
            var = ssb.tile([1, RB], F32, tag="var")
            nc.vector.tensor_mul(musq[:], mv[:, 0:RB], mv[:, 0:RB])
            nc.vector.tensor_sub(var[:], mv[:, RB:2 * RB], musq[:])
            eps_t = ssb.tile([1, 1], F32, tag="eps")
            nc.gpsimd.memset(eps_t[:], LN_EPS)
            std = ssb.tile([1, RB], F32, tag="musq", name="std")
            rstd = ssb.tile([1, RB], F32, tag="var", name="rstd")
            nc.scalar.activation(std[:], var[:], AF.Sqrt, bias=eps_t[:])
            nc.vector.reciprocal_approx_fast(rstd[:], std[:])
            mu_b = ssb.tile([1, RB], BF16, tag="mub")
            rstd_b = ssb.tile([1, RB], BF16, tag="rstdb")
            nc.vector.tensor_copy(mu_b[:], mv[:, 0:RB])
            nc.vector.tensor_copy(rstd_b[:], rstd[:])
            mu_s = sb3.tile([128, RB], BF16, tag="mus")
            rs_s = sb3.tile([128, RB], BF16, tag="rss")
            nc.gpsimd.partition_broadcast(mu_s[:], mu_b[:])
            nc.gpsimd.partition_broadcast(rs_s[:], rstd_b[:])
            gated = sb3.tile([128, 3, RB], BF16, tag="gated")
            for ct in range(3):
                d1 = sb3.tile([128, RB], BF16, tag="d1")
                d2 = sb3.tile([128, RB], BF16, tag="d2")
                nc.vector.tensor_sub(d1[:], ao_sb[:, ct, sl], mu_s[:])
                nc.vector.tensor_mul(d2[:], d1[:], rs_s[:])
                nc.vector.tensor_mul(gated[:, ct, :], d2[:], ut_sb[:, ct, sl])
            gin = dram.tile([3, 128, RB], BF16, tag="gin")
            gout = dram.tile([2, 3, 128, RB], BF16, tag="gout")
            nc.gpsimd.dma_start(out=gin.rearrange("p i j -> i p j"),
                                in_=gated[:])
            nc.gpsimd.collective_compute(
                "AllGather", mybir.AluOpType.bypass, replica_groups=pairs,
                ins=[gin.opt()], outs=[gout.opt()])
            gfull = sb3.tile([128, 6, RB], BF16, tag="gfull")
            nc.sync.dma_start(out=gfull[:],
                              in_=gout.rearrange("r p i j -> i (r p) j"))
            for ctp in range(3):
                po = opo.tile([128, RB], F32, tag="po")
                for ct in range(6):
                    nc.tensor.matmul(
                        po[:], wout_sb[:, ct, ctp * 128:(ctp + 1) * 128],
                        gfull[:, ct, :], start=(ct == 0), stop=(ct == 5))
                rt_t = sb3.tile([128, RB], F32, tag="resid")
                nc.sync.dma_start(out=rt_t[:], in_=residT_r[:, ctp, sl])
                o_t = sb3.tile([128, RB], F32, tag="osb")
                nc.vector.tensor_add(o_t[:], po[:], rt_t[:])
                nc.gpsimd.dma_start(out=out_r[:, ctp, sl], in_=o_t[:])

        # ------------- emission ---------------------------------------
        with tc.tile_pool(name="p1psum", bufs=2, space="PSUM") as pp:
            for nb in range(4):
                proj_block(pp, nb)
                if nb < 3:
                    attn_block(nb)
        stp = es.enter_context(tc.tile_pool(name="stp", bufs=1, space="PSUM"))
        opo = es.enter_context(tc.tile_pool(name="opo", bufs=2, space="PSUM"))
        sb3 = es.enter_context(tc.tile_pool(name="p3sb", bufs=1))
        ssb = es.enter_context(tc.tile_pool(name="p3small", bufs=1))
        attn_pair(3, 0)
        epilogue(0, stp, opo, sb3, ssb)
        attn_pair(3, 1)
        epilogue(1, stp, opo, sb3, ssb)
        attn_pair(3, 2)
        epilogue(2, stp, opo, sb3, ssb)
        epilogue(3, stp, opo, sb3, ssb)

        if dbg is not None:
            nc.gpsimd.dma_start(out=dbg["qt_d"], in_=qt_sb[:])
            nc.gpsimd.dma_start(out=dbg["kt_d"], in_=kt_sb[:])
            nc.gpsimd.dma_start(out=dbg["v_d"], in_=v_sb[:])
            nc.gpsimd.dma_start(out=dbg["ut_d"], in_=ut_sb[:])
            nc.gpsimd.dma_start(out=dbg["ao_d"], in_=ao_sb[:])


# ---------------------------------------------------------------------------
# host side
# ---------------------------------------------------------------------------

def prep_inputs(x, attn_mask, W_proj, b_proj, ln_gamma, ln_beta, W_out, b_out):
    x = np.asarray(x, dtype=np.float32)
    W_proj = np.asarray(W_proj, dtype=np.float32)
    b_proj = np.asarray(b_proj, dtype=np.float32)
    ln_gamma = np.asarray(ln_gamma, dtype=np.float32)
    ln_beta = np.asarray(ln_beta, dtype=np.float32)
    W_out = np.asarray(W_out, dtype=np.float32)
    b_out = np.asarray(b_out, dtype=np.float32)

    tril = np.tril(np.ones((S, S), dtype=bool))
    am = np.asarray(attn_mask)
    if not all(np.array_equal(am[b], tril) for b in range(am.shape[0])):
        raise ValueError("kernel specialized for causal attn_mask")
    if np.any(b_proj != 0) or np.any(ln_beta != 0):
        raise ValueError("kernel specialized for zero b_proj / ln_beta")

    bf = ml_dtypes.bfloat16
    cos, sin = _rope_tables()                          # [S, 64]
    cosT = np.ascontiguousarray(cos.T)                 # [64, S]
    # source-side rotate factor g: row d carries the factor applied to
    # Q[d] BEFORE the 32-block shift: +sin for d<32, -sin for d>=32
    sinfT = np.ascontiguousarray(sin.T).copy()
    sinfT[32:64] *= -1.0
    cosT2 = np.vstack([cosT, cosT]).astype(bf)         # [128, S]
    sinfT2 = np.vstack([sinfT, sinfT]).astype(bf)

    ii = np.arange(128)[:, None]
    mf = (np.arange(RB)[None, :] >= ii).astype(np.float32)   # [128, 512]
    maskf = mf.astype(bf)
    maskd2 = np.concatenate([mf[:, 0:256], mf[:, 0:128]],
                            axis=1).astype(bf)               # [128, 384]
    ones_k = np.ones((128, 1), dtype=bf)

    Wg = (ln_gamma[:, None] * W_out).astype(np.float32)
    U_c, V_c, Q_c, K_c = 0, HID, 2 * HID, 3 * HID

    in_maps = []
    for c in range(N_CORES):
        b, hh = c // 2, c % 2
        heads = range(NH * hh, NH * hh + NH)
        qcols = np.concatenate(
            [np.arange(Q_c + h * D, Q_c + (h + 1) * D) for h in heads])
        kcols = qcols - Q_c + K_c
        vcols = qcols - Q_c + V_c
        ucols = np.arange(U_c + hh * 384, U_c + (hh + 1) * 384)
        w_qku = np.concatenate(
            [W_proj[:, qcols], W_proj[:, kcols], W_proj[:, ucols]],
            axis=1).astype(bf)
        wv = W_proj[:, vcols].astype(bf)
        w_out_half = Wg[:, hh * 384:(hh + 1) * 384].astype(bf)
        xTb = x[b].T                                   # [768, 2048]
        residT = (xTb[hh * 384:(hh + 1) * 384, :]
                  + b_out[hh * 384:(hh + 1) * 384, None]).astype(np.float32)
        in_maps.append(dict(
            xT=np.ascontiguousarray(xTb).astype(bf),
            w_qku=np.ascontiguousarray(w_qku),
            wv=np.ascontiguousarray(wv),
            w_out=np.ascontiguousarray(w_out_half),
            cosT2=cosT2, sinfT2=sinfT2, maskf=maskf, maskd2=maskd2,
            ones_k=ones_k,
            residT=np.ascontiguousarray(residT),
        ))
    return in_maps


def assemble(results, B=4):
    full = np.empty((B, S, HID), dtype=np.float32)
    for c in range(N_CORES):
        b, hh = c // 2, c % 2
        full[b, :, hh * 384:(hh + 1) * 384] = results[c]["out"].T
    return full


_NC_CACHE = {}


def get_nc(ndev=N_CORES):
    if ndev not in _NC_CACHE:
        pairs = [[i, i + 1] for i in range(0, ndev, 2)]
        _NC_CACHE[ndev] = build_nc(ndev, pairs)
    return _NC_CACHE[ndev]


def kernel(**inputs):
    in_maps = prep_inputs(**inputs)
    nc = get_nc(N_CORES)
    res = bass_utils.run_bass_kernel_spmd(
        nc, in_maps, core_ids=list(range(N_CORES)))
    return assemble(res.results)
